# revision 7
# baseline (speedup 1.0000x reference)
"""ARMA-style GNN message passing on 8 TRN2 NeuronCores — push/ReduceScatter.

Reference computation (per layer, 7 layers):
    m   = h @ W                                  [N, CH]
    agg = segment_sum(w[:,None] * m[dst], src)   [N, CH]
    h'  = relu(agg + h @ V + b)
then logits = h @ Wd + bd.

Strategy vs the AllGather/pull baseline:
  - The edge weight is separable: w_e = a[src] * b[dst] with
    a = deg_out^-1/2, b = deg_in^-1/2. Each core owns the edges whose dst
    it owns: it computes m~ = b (.) (h @ W) for its own nodes, gathers
    message rows from its LOCAL m~ table (one indirect DMA per 128-edge
    block), and accumulates them into a partial aggregate table over all
    10240 padded global rows via selection-matrix matmuls (a[src] baked
    into the selection weights, f32 PSUM accumulation).
  - Two row-half ReduceScatters per layer sum the partial tables and hand
    each core the aggregate rows for its own nodes (output 0.65MB ->
    ~31us each, vs a 10.5MB-output AllGather at ~239us); each collective
    overlaps the other half's gather/matmul work.
  - Nodes are permuted within each core (host-side greedy balancing with
    one spill block per core) so every (processing core, global block)
    edge bucket fits 2 x 128 slots (spill: 3), keeping gather padding low.
  - DMAs are batched (multi-block staging tiles, one descriptor-heavy copy
    each) to amortize the fixed per-DMA HWDGE overhead; layer weights are
    streamed from DRAM to fit SBUF.
  - h@V+agg+bias+relu and PE transposes run per half behind the other
    half's collective. Final dense layer as in the baseline. All matmuls
    bf16 with f32 PSUM accumulation.
"""
import numpy as np
import ml_dtypes

import concourse.bass as bass
import concourse.tile as tile
import concourse.mybir as mybir
from concourse import library_config
from concourse.vector_clock import ScopedClock
from concourse.bass_utils import run_bass_kernel_spmd
from concourse.masks import make_identity

# ---------------------------------------------------------------- constants
N_NODES = 10000
N_EDGES = 160000
IN_F = 256
CH = 512
N_LABELS = 1440
NCORES = 8
NPC = N_NODES // NCORES      # 1250 nodes per core
P = 128
NBL = 10                     # node blocks per core (10*128 = 1280)
NPAD = NBL * P               # padded nodes per core
NGB = NCORES * NBL           # global node blocks
NHALF = NBL // 2             # node blocks per RS half
HROWS = NHALF * P            # rows per core per RS half
NLAYERS = 7
KG1 = IN_F // P              # 2 contraction blocks in layer 1
KGC = CH // P                # 4 contraction blocks in layers 2..7
FIN_CHUNK = 480              # 1440 = 3 * 480, fits one PSUM bank in f32
BLK_CAP = 125
NDENSE = 24                  # buckets routed dense (PE) instead of gather                # nodes per block (balance headroom, 10*125=1250)

BF = mybir.dt.bfloat16
F32 = mybir.dt.float32
BFNP = ml_dtypes.bfloat16


# ------------------------------------------------------- walrus workarounds
def _patched_drain_and_barrier(self, tick_clock, wait_clock):
    # This walrus build rejects >1-2 sync waits on one TPB_CTRL; put the
    # kernel-tail drain's waits on separate preceding SP nops instead.
    nc = self.nc
    probe = nc.sync.nop(nofuse=True, hint="drain_waits")
    wait_clock.add_sem_waits(probe.ins, ScopedClock({None: tick_clock.global_clock}))
    si = probe.ins.sync_info
    waits = list(si.on_wait) if si is not None else []
    if len(waits) > 1:
        si.on_wait = waits[:1]
        for i in range(1, len(waits)):
            n2 = nc.sync.nop(nofuse=True, hint=f"drain_waits_{i}")
            n2.ins.sync_info = mybir.SyncInfo(on_wait=[waits[i]], on_update=[])
    nc.sync.drain()
    nc.all_engine_barrier()
    assert self.sems is not None
    popped = nc._tile_sem_poison_stack.pop()
    assert popped is self._sem_poison
    nc.clear_and_free_semaphores(list(self.sems.allocated().values()))
    nc.all_engine_barrier()


tile.TileContext._drain_and_barrier = _patched_drain_and_barrier


def _split_excess_waits(nc, limit=1):
    # Same ISA restriction for ordinary instructions: hoist excess sync
    # waits onto injected same-engine nops placed just before.
    for func in nc.m.functions:
        for bb in func.blocks:
            out = []
            for ins in bb.instructions:
                si = ins.sync_info
                if si is not None and si.on_wait and len(si.on_wait) > limit:
                    waits = list(si.on_wait)
                    excess, keep = waits[:-limit], waits[-limit:]
                    for i in range(0, len(excess), limit):
                        out.append(mybir.InstNoOp(
                            name=f"{ins.name}_xw{i}",
                            engine=ins.engine,
                            ins=[], outs=[],
                            sync_info=mybir.SyncInfo(
                                on_wait=excess[i:i + limit], on_update=[]),
                        ))
                    si.on_wait = keep
                out.append(ins)
            bb.instructions[:] = out


# ------------------------------------------------------------- host prep
def _balance_blocks(vmat):
    """Assign the NPC nodes (rows of vmat [NPC, NCORES] = per-core in-bucket
    edge counts) to NBL blocks of <= BLK_CAP nodes so that per-core block
    loads fit 2*P slots (the last block is a spill allowed 3*P). Greedy +
    swap refinement. Returns pos[NPC] = padded slot index."""
    limit = np.full((NBL, 1), 2 * P, np.int64)
    limit[NBL - 1] = 3 * P

    order = np.argsort(-vmat.max(axis=1), kind="stable")
    loads = np.zeros((NBL, NCORES), np.int64)
    counts = np.zeros(NBL, np.int64)
    blk = np.zeros(NPC, np.int64)
    for n in order:
        v = vmat[n]
        feas = np.nonzero(counts < BLK_CAP)[0]
        peak = (loads[feas] + v).max(axis=1)
        bsel = feas[np.lexsort((counts[feas], peak))[0]]
        loads[bsel] += v
        counts[bsel] += 1
        blk[n] = bsel

    def badness(lo, bs=None):
        lim = limit if bs is None else limit[bs]
        return np.maximum(lo - lim, 0).sum()

    members = [list(np.nonzero(blk == b)[0]) for b in range(NBL)]
    for _ in range(3000):
        bad = badness(loads)
        if bad == 0:
            break
        b1, p = np.unravel_index(np.argmax(loads - limit), loads.shape)
        m1 = np.array(members[b1])
        n1s = m1[np.argsort(-vmat[m1, p])[:6]]
        best = None
        for n1 in n1s:
            v1 = vmat[n1]
            for b2 in range(NBL):
                if b2 == b1:
                    continue
                m2 = np.array(members[b2])
                d = v1[None, :] - vmat[m2]
                nl1 = loads[b1][None, :] - d
                nl2 = loads[b2][None, :] + d
                nb = (np.maximum(nl1 - limit[b1], 0).sum(1)
                      + np.maximum(nl2 - limit[b2], 0).sum(1)
                      + bad - badness(loads[b1], b1) - badness(loads[b2], b2))
                j = int(np.argmin(nb))
                if best is None or nb[j] < best[0]:
                    best = (nb[j], int(n1), int(m2[j]), b2)
        if best is None or best[0] >= bad:
            break
        _, n1, n2, b2 = best
        d = vmat[n1] - vmat[n2]
        loads[b1] -= d
        loads[b2] += d
        members[b1].remove(n1)
        members[b2].remove(n2)
        members[b1].append(n2)
        members[b2].append(n1)
        blk[n1], blk[n2] = b2, b1

    pos = np.zeros(NPC, np.int64)
    fill = np.zeros(NBL, np.int64)
    for n in range(NPC):
        pos[n] = blk[n] * P + fill[blk[n]]
        fill[blk[n]] += 1
    return pos


def _prep_edges(src, dst):
    """Permute nodes per owning core for bucket balance; partition edges by
    dst owner; bucket by global src block; build per-core gather index and
    selection tables plus the degree-scale vectors.

    Returns (schedule, tables, None); schedule = (kgb tuple,) is the static
    program shape, tables carry the per-core input arrays."""
    src = np.asarray(src).astype(np.int64)
    dst = np.asarray(dst).astype(np.int64)
    deg_out = np.maximum(np.bincount(src, minlength=N_NODES), 1.0)
    deg_in = np.maximum(np.bincount(dst, minlength=N_NODES), 1.0)
    a_n = (1.0 / np.sqrt(deg_out)).astype(np.float32)   # per src node
    b_n = (1.0 / np.sqrt(deg_in)).astype(np.float32)    # per dst node

    sown = src // NPC            # owner of src (bucket side)
    down = dst // NPC            # owner of dst (processing core)

    pos_all = np.zeros(N_NODES, np.int64)
    for q in range(NCORES):
        vmat = np.zeros((NPC, NCORES), np.int64)
        mask = sown == q
        np.add.at(vmat, (src[mask] - q * NPC, down[mask]), 1)
        pos_all[q * NPC:(q + 1) * NPC] = q * NPAD + _balance_blocks(vmat)

    gpos = pos_all[src]                              # padded-global src row
    gb = (gpos // NPAD) * NBL + (gpos % NPAD) // P   # global block 0..79
    gcol = gpos % P                                  # column within block
    dloc = pos_all[dst] % NPAD                       # local padded dst row

    order = np.lexsort((gb, down))
    down_s, gb_s = down[order], gb[order]
    gcol_s, dloc_s = gcol[order], dloc[order]
    w_s = a_n[src[order]]
    counts = np.zeros((NCORES, NGB), np.int64)
    np.add.at(counts, (down_s, gb_s), 1)
    kgb_all = [max(1, int(-(-counts[:, g].max() // P))) for g in range(NGB)]
    # hybrid: route the heaviest NDENSE buckets through dense block-matmuls
    # (PE) instead of indirect gathers (Pool); pick per half-and-peer evenly
    order_d = sorted(range(NGB), key=lambda g: (-kgb_all[g], g % NBL, g))
    dense = [False] * NGB
    for g in order_d[:NDENSE]:
        dense[g] = True
    kgb = tuple(0 if dense[g] else kgb_all[g] for g in range(NGB))
    kgb_full = tuple(kgb_all)
    neb = sum(kgb)

    starts = np.zeros((NCORES, NGB), np.int64)
    flat = counts.ravel().cumsum()
    starts.ravel()[1:] = flat[:-1]

    ndense = sum(dense)
    idx_tabs, sel_tabs, dw_tabs = [], [], []
    for p in range(NCORES):
        idx_t = np.zeros((P, max(1, neb)), np.int32)  # slot i -> [i%128, blk]
        sel_t = np.zeros((P, max(1, neb) * P), np.float32)
        dw_t = np.zeros((P, max(1, ndense) * NPAD), np.float32)
        col = 0
        dj = 0
        for g in range(NGB):
            s0, cnt = starts[p, g], counts[p, g]
            d_rows = dloc_s[s0:s0 + cnt]
            cols = gcol_s[s0:s0 + cnt]
            ww = w_s[s0:s0 + cnt]
            if dense[g]:
                dw = np.zeros((NPAD, P), np.float32)
                np.add.at(dw, (d_rows, cols), ww)
                # pack [d, s] -> [d%128, lb*128 + s]
                dw_t[:, dj * NPAD:(dj + 1) * NPAD] = (
                    dw.reshape(NBL, P, P).transpose(1, 0, 2).reshape(P, NPAD))
                dj += 1
                continue
            for k in range(kgb[g]):
                lo, hi = k * P, min((k + 1) * P, cnt)
                if hi > lo:
                    sl = np.arange(lo, hi) - lo
                    idx_t[sl, col] = d_rows[lo:hi]
                    sel_t[sl, col * P + cols[lo:hi]] = ww[lo:hi]
                col += 1
        idx_tabs.append(idx_t)
        sel_tabs.append(sel_t.astype(BFNP))
        dw_tabs.append(dw_t.astype(BFNP))

    bvecs = []
    for p in range(NCORES):
        bv = np.zeros((P, NBL), np.float32)
        nodes = np.arange(p * NPC, (p + 1) * NPC)
        posl = pos_all[nodes] % NPAD
        bv[posl % P, posl // P] = b_n[nodes]
        bvecs.append(bv)

    schedule = (kgb, tuple(dense))
    tables = dict(idx_tabs=idx_tabs, sel_tabs=sel_tabs, dw_tabs=dw_tabs,
                  bvecs=bvecs, pos_all=pos_all)
    return schedule, tables, None


def _pack_lhsT(xT, kg):
    """[kg*128, NPAD] -> [128, kg*NPAD] (partition-major kg blocks)."""
    return np.ascontiguousarray(
        xT.reshape(kg, P, NPAD).transpose(1, 0, 2).reshape(P, kg * NPAD))


def _pack_rhs(Wm, kg, n):
    """[kg*128, n] -> [128, kg*n]."""
    return np.ascontiguousarray(
        Wm.reshape(kg, P, n).transpose(1, 0, 2).reshape(P, kg * n))


# ------------------------------------------------------------- device build
def _build(schedule, repeat=1):
    (kgb, dense) = schedule
    ndense = sum(dense)
    dseq = [g for g in range(NGB) if dense[g]]
    dpos = {g: j for j, g in enumerate(dseq)}
    neb = sum(kgb)
    cum = np.concatenate([[0], np.cumsum(kgb)]).astype(int)
    # gather/stage units, half-major: (peer q, gb base, col offset, blocks)
    units = {0: [], 1: []}
    for hf in range(2):
        for q in range(NCORES):
            g0 = q * NBL + hf * NHALF
            units[hf].append((q, g0, int(cum[g0]),
                              int(cum[g0 + NHALF] - cum[g0])))

    nc = bass.Bass("TRN2", target_bir_lowering=False, debug=False,
                   num_devices=NCORES)

    def din(name, shape, dt):
        return nc.dram_tensor(name, shape, dt, kind="ExternalInput").ap()

    xT = din("xT", [P, KG1 * NPAD], BF)
    idx = din("idx", [P, max(1, neb)], mybir.dt.int32)
    dwd = din("dwd", [P, max(1, ndense) * NPAD], BF)
    sel = din("sel", [P, neb * P], BF)
    w1 = din("w1", [P, KG1 * CH], BF)
    v1 = din("v1", [P, KG1 * CH], BF)
    wk = din("wk", [P, 6 * KGC * CH], BF)
    vk = din("vk", [P, 6 * KGC * CH], BF)
    wd = din("wd", [P, KGC * N_LABELS], BF)
    ballT = din("ballT", [P, NLAYERS * KGC], F32)
    bdr = din("bdr", [P, N_LABELS], F32)
    bvec = din("bvec", [P, NBL], F32)
    out = nc.dram_tensor("out", [NPAD, N_LABELS], F32, kind="ExternalOutput").ap()

    nreg_cache = {}

    def nreg(v):
        if v not in nreg_cache:
            nreg_cache[v] = nc.gpsimd.to_reg(v)
        return nreg_cache[v]

    with tile.TileContext(nc) as tc:
        with (
            tc.tile_pool(name="const", bufs=1) as cp,
            tc.tile_pool(name="wkv", bufs=4) as wp,
            tc.tile_pool(name="dwp", bufs=2) as dwp,
            tc.tile_pool(name="ht", bufs=2) as htp,
            tc.tile_pool(name="msg", bufs=2) as msgp,
            tc.tile_pool(name="part", bufs=2) as pstp,
            tc.tile_pool(name="aggs", bufs=2) as aggp,
            tc.tile_pool(name="hvst", bufs=2) as hvp,
            tc.tile_pool(name="outs", bufs=1) as op,
            tc.tile_pool(name="psm", bufs=2, space="PSUM") as psm,
            tc.tile_pool(name="psagg", bufs=4, space="PSUM") as psagg,
            tc.tile_pool(name="pshv", bufs=2, space="PSUM") as pshv,
            tc.tile_pool(name="dram", bufs=2, space="DRAM") as dmp,
            tc.tile_pool(name="dramrs", bufs=4, space="DRAM") as drs,
            tc.tile_pool(name="dramro", bufs=4, space="DRAM") as dro,
        ):
            # ---- constants to SBUF
            xT_t = cp.tile([P, KG1 * NPAD], BF)
            nc.sync.dma_start(xT_t[:], xT[:])
            idx_t = cp.tile([P, max(1, neb)], mybir.dt.int32)
            nc.sync.dma_start(idx_t[:], idx[:])
            sel_t = cp.tile([P, max(1, neb) * P], BF)
            nc.sync.dma_start(sel_t[:], sel[:])
            w1_t = cp.tile([P, KG1 * CH], BF)
            nc.sync.dma_start(w1_t[:], w1[:])
            v1_t = cp.tile([P, KG1 * CH], BF)
            nc.sync.dma_start(v1_t[:], v1[:])
            wd_t = cp.tile([P, KGC * N_LABELS], BF)
            nc.sync.dma_start(wd_t[:], wd[:])
            ballT_t = cp.tile([P, NLAYERS * KGC], F32)
            nc.sync.dma_start(ballT_t[:], ballT[:])
            bdr_t = cp.tile([P, N_LABELS], F32)
            nc.sync.dma_start(bdr_t[:], bdr[:])
            bvec_t = cp.tile([P, NBL], F32)
            nc.sync.dma_start(bvec_t[:], bvec[:])
            ident = cp.tile([P, P], BF)
            make_identity(nc, ident[:])
            msbres = cp.tile([P, NBL * CH], BF)      # resident m~ staging

            def emit_units(hf, mdram, mid_cb=None):
                """Gather + selection matmuls + staging for one RS half.
                mid_cb (if set) is emitted after the 4th unit so the Pool
                queue dispatches the next half's gathers before parking on
                the collective. Returns the half's rs_in DRAM tile."""
                rs_in = drs.tile([NCORES * HROWS, CH], BF, tag=f"rsin{hf}")
                for ui, (q, g0, u0, nbu) in enumerate(units[hf]):
                    if ui == 4 and mid_cb is not None:
                        mid_cb()
                    msg = msgp.tile([P, max(1, nbu) * CH], BF, tag="msg")
                    for k in range(nbu):
                        nc.gpsimd.indirect_dma_start(
                            out=msg[:, k * CH:(k + 1) * CH], out_offset=None,
                            in_=mdram[:],
                            in_offset=bass.IndirectOffsetOnAxis(
                                ap=idx_t[:, u0 + k:u0 + k + 1], axis=0))
                    pst = pstp.tile([P, NHALF * CH], BF, tag="pst")
                    colbase = {}
                    c_ = u0
                    for b_ in range(NHALF):
                        colbase[b_] = c_
                        if not dense[g0 + b_]:
                            c_ += kgb[g0 + b_]
                    # dense buckets first: they only need SBUF m~, so PE can
                    # run them while this unit's gathers are still in flight
                    for bb in sorted(range(NHALF),
                                     key=lambda b_: not dense[g0 + b_]):
                        g = g0 + bb
                        agg_ps = psagg.tile([P, CH], F32, tag="agg")
                        col = colbase[bb]
                        if dense[g]:
                            dw_t = dwp.tile([P, NPAD], BF, name="dw_t",
                                            tag="dw")
                            nc.sync.dma_start(
                                dw_t[:],
                                dwd[:, dpos[g] * NPAD:(dpos[g] + 1) * NPAD])
                            for lb in range(NBL):
                                nc.tensor.matmul(
                                    agg_ps[:],
                                    dw_t[:, lb * P:(lb + 1) * P],
                                    msbres[:, lb * CH:(lb + 1) * CH],
                                    start=(lb == 0), stop=(lb == NBL - 1))
                        else:
                            for k in range(kgb[g]):
                                nc.tensor.matmul(
                                    agg_ps[:],
                                    sel_t[:, col * P:(col + 1) * P],
                                    msg[:, (col - u0) * CH:(col - u0 + 1) * CH],
                                    start=(k == 0), stop=(k == kgb[g] - 1))
                                col += 1
                        if bb % 2 == 0:
                            nc.vector.tensor_copy(
                                pst[:, bb * CH:(bb + 1) * CH], agg_ps[:])
                        else:
                            nc.scalar.activation(
                                pst[:, bb * CH:(bb + 1) * CH], agg_ps[:],
                                mybir.ActivationFunctionType.Copy)
                    nc.sync.dma_start(
                        rs_in[q * HROWS:(q + 1) * HROWS, :].rearrange(
                            "(n p) e -> p n e", p=P),
                        pst[:, :].rearrange("p (n e) -> p n e", e=CH))
                return rs_in

            def emit_rs(rs_in, hf):
                rs_out = dro.tile([HROWS, CH], BF, tag=f"rsout{hf}")
                nc.gpsimd.collective_compute(
                    "ReduceScatter", mybir.AluOpType.add,
                    replica_groups=[list(range(NCORES))],
                    ins=[rs_in[:].opt()], outs=[rs_out[:].opt()])
                return rs_out

            def emit_combine(hf, rs_out, l, kg, lhsT_t, vt, hT_next,
                             wt_next, mdram_next):
                """hT' = relu((hV)^T + agg^T + b) built transposed and fully
                decoupled from the collective on the PE side: (hV)^T goes to
                SBUF via PSUM copies, agg^T arrives via XBAR-transposing DMA
                reads on the SP queue (ordered after its rs_in writes), and
                the combine itself is one wide DVE add + per-chunk Act
                relu+bias writing hT_next directly. Then this half's m~ for
                the NEXT layer is produced so it hides behind the other
                half's collective."""
                h0 = hf * HROWS                      # node offset of the half
                hvs = hvp.tile([P, KGC * HROWS], BF, tag="hvs")
                for cc in range(KGC):
                    for (t0, tl) in ((0, 4 * P), (4 * P, P)):
                        h_ps = pshv.tile([P, 4 * P], F32, tag="hv")
                        for g in range(kg):
                            nc.tensor.matmul(
                                h_ps[:, :tl],
                                vt[:, g * CH + cc * P:g * CH + (cc + 1) * P],
                                lhsT_t[:, g * NPAD + h0 + t0:
                                       g * NPAD + h0 + t0 + tl],
                                start=(g == 0), stop=(g == kg - 1))
                        nc.scalar.activation(
                            hvs[:, cc * HROWS + t0:cc * HROWS + t0 + tl],
                            h_ps[:, :tl], mybir.ActivationFunctionType.Copy)
                aggT = aggp.tile([P, KGC * HROWS], BF, tag="aggT")
                for cc in range(KGC):
                    nc.sync.dma_start_transpose(
                        aggT[:, cc * HROWS:(cc + 1) * HROWS],
                        rs_out[:, cc * P:(cc + 1) * P])
                nc.vector.tensor_add(aggT[:], aggT[:], hvs[:])
                for cc in range(KGC):
                    nc.scalar.activation(
                        hT_next[:, cc * NPAD + h0:cc * NPAD + h0 + HROWS],
                        aggT[:, cc * HROWS:(cc + 1) * HROWS],
                        mybir.ActivationFunctionType.Relu,
                        bias=ballT_t[:, l * KGC + cc:l * KGC + cc + 1])
                if wt_next is None:
                    return
                msb = msbres[:, :]
                for bb in range(NHALF):
                    b = hf * NHALF + bb
                    m_ps = psm.tile([P, CH], F32, tag="m")
                    for g in range(KGC):
                        nc.tensor.matmul(
                            m_ps[:],
                            hT_next[:, g * NPAD + b * P:g * NPAD + (b + 1) * P],
                            wt_next[:, g * CH:(g + 1) * CH],
                            start=(g == 0), stop=(g == KGC - 1))
                    nc.scalar.activation(
                        msb[:, b * CH:(b + 1) * CH], m_ps[:],
                        mybir.ActivationFunctionType.Copy,
                        scale=bvec_t[:, b:b + 1])
                nc.sync.dma_start(
                    mdram_next[hf * HROWS:(hf + 1) * HROWS, :].rearrange(
                        "(n p) e -> p n e", p=P),
                    msb[:, hf * NHALF * CH:(hf * NHALF + NHALF) * CH].rearrange(
                        "p (n e) -> p n e", e=CH))

            def load_v(l):
                if l == 0:
                    return v1_t[:, :]
                vt_t = wp.tile([P, KGC * CH], BF, name="vt_t", tag="vt")
                nc.sync.dma_start(
                    vt_t[:], vk[:, (l - 1) * KGC * CH:l * KGC * CH])
                return vt_t[:, :]

            def load_w(l):
                if l == 0:
                    return w1_t[:, :]
                wt_t = wp.tile([P, KGC * CH], BF, name="wt_t", tag="wt")
                nc.sync.dma_start(
                    wt_t[:], wk[:, (l - 1) * KGC * CH:l * KGC * CH])
                return wt_t[:, :]

            def emit_layer(l, hT_cur, mdram):
                """Process layer l given its m~ table; produce hT_next and
                (except at the last layer) the next layer's m~ table."""
                kg = KG1 if l == 0 else KGC
                lhsT_t = xT_t if l == 0 else hT_cur
                vt = load_v(l)
                wt_next = load_w(l + 1) if l + 1 < NLAYERS else None
                mdram_next = None
                if wt_next is not None:
                    mdram_next = dmp.tile([NPAD, CH], BF, name="mdram_n",
                                          tag="mdram")

                hT_next = htp.tile([P, KGC * NPAD], BF, tag="hT")
                rs_in0 = emit_units(0, mdram)
                box = {}

                def mid():
                    box["rs_out0"] = emit_rs(rs_in0, 0)

                rs_in1 = emit_units(1, mdram, mid_cb=mid)
                rs_out1 = emit_rs(rs_in1, 1)
                emit_combine(0, box["rs_out0"], l, kg, lhsT_t, vt, hT_next,
                             wt_next, mdram_next)
                emit_combine(1, rs_out1, l, kg, lhsT_t, vt, hT_next,
                             wt_next, mdram_next)
                return hT_next, mdram_next

            def emit_m0():
                """Prologue: m~ for layer 0 from xT and W1."""
                mdram = dmp.tile([NPAD, CH], BF, tag="mdram")
                msb = msbres[:, :]
                wt = load_w(0)
                for b in range(NBL):
                    m_ps = psm.tile([P, CH], F32, tag="m")
                    for g in range(KG1):
                        nc.tensor.matmul(
                            m_ps[:],
                            xT_t[:, g * NPAD + b * P:g * NPAD + (b + 1) * P],
                            wt[:, g * CH:(g + 1) * CH],
                            start=(g == 0), stop=(g == KG1 - 1))
                    nc.scalar.activation(
                        msb[:, b * CH:(b + 1) * CH], m_ps[:],
                        mybir.ActivationFunctionType.Copy,
                        scale=bvec_t[:, b:b + 1])
                nc.sync.dma_start(
                    mdram[:, :].rearrange("(n p) e -> p n e", p=P),
                    msb[:, :].rearrange("p (n e) -> p n e", e=CH))
                return mdram

            def emit_final(hT_cur, rep):
                for b in range(NBL):
                    o_sb = op.tile([P, N_LABELS], F32, tag="o")
                    fps = []
                    for c in range(3):
                        fin_ps = psagg.tile([P, FIN_CHUNK], F32, tag="agg")
                        fps.append(fin_ps)
                    for g in range(KGC):
                        for c in range(3):
                            nc.tensor.matmul(
                                fps[c][:],
                                hT_cur[:, g * NPAD + b * P:g * NPAD + (b + 1) * P],
                                wd_t[:, g * N_LABELS + c * FIN_CHUNK:
                                     g * N_LABELS + (c + 1) * FIN_CHUNK],
                                start=(g == 0), stop=(g == KGC - 1))
                    for c in range(3):
                        sl = slice(c * FIN_CHUNK, (c + 1) * FIN_CHUNK)
                        nc.vector.tensor_add(fps[c][:], fps[c][:], bdr_t[:, sl])
                        nc.scalar.activation(
                            o_sb[:, sl], fps[c][:],
                            mybir.ActivationFunctionType.Copy)
                    if rep == repeat - 1:
                        nc.sync.dma_start(out[b * P:(b + 1) * P, :], o_sb[:])

            for rep in range(repeat):
                hT_cur, mdram = None, emit_m0()
                for l in range(NLAYERS):
                    hT_cur, mdram = emit_layer(l, hT_cur, mdram)
                emit_final(hT_cur, rep)

    _split_excess_waits(nc)
    return nc


# ------------------------------------------------------------- entry point
def kernel(x, src, dst, W1, V1, b1, Wk, Vk, bk, Wd, bd, _repeat=1, _nc_cache={}):
    x = np.asarray(x, np.float32)
    schedule, tables, _ = _prep_edges(src, dst)

    key = (schedule, _repeat)
    if key not in _nc_cache:
        _nc_cache[key] = _build(schedule, repeat=_repeat)
    nc = _nc_cache[key]

    # weights (replicated, host-packed)
    w1p = _pack_rhs(np.asarray(W1, np.float32), KG1, CH).astype(BFNP)
    v1p = _pack_rhs(np.asarray(V1, np.float32), KG1, CH).astype(BFNP)
    wkp = np.concatenate(
        [_pack_rhs(np.asarray(Wk[i], np.float32), KGC, CH) for i in range(6)],
        axis=1).astype(BFNP)
    vkp = np.concatenate(
        [_pack_rhs(np.asarray(Vk[i], np.float32), KGC, CH) for i in range(6)],
        axis=1).astype(BFNP)
    wdp = _pack_rhs(np.asarray(Wd, np.float32), KGC, N_LABELS).astype(BFNP)
    ballv = np.stack(
        [np.asarray(b1, np.float32)] + [np.asarray(bk[i], np.float32)
                                        for i in range(6)])       # [7, CH]
    # transposed per-partition bias: [c_in, l*KGC + cc] = bias[l][cc*128+c_in]
    ballTp = np.ascontiguousarray(
        ballv.reshape(NLAYERS, KGC, P).transpose(2, 0, 1).reshape(P, NLAYERS * KGC))
    bdp = np.broadcast_to(np.asarray(bd, np.float32), (P, N_LABELS)).copy()

    pos_all = tables["pos_all"]
    in_maps = []
    for p in range(NCORES):
        xp = np.zeros((NPAD, IN_F), np.float32)
        nodes = np.arange(p * NPC, (p + 1) * NPC)
        xp[pos_all[nodes] % NPAD] = x[nodes]
        xTp = _pack_lhsT(np.ascontiguousarray(xp.T), KG1).astype(BFNP)
        in_maps.append({
            "xT": xTp, "idx": tables["idx_tabs"][p],
            "sel": tables["sel_tabs"][p], "dwd": tables["dw_tabs"][p],
            "w1": w1p, "v1": v1p, "wk": wkp, "vk": vkp, "wd": wdp,
            "ballT": ballTp, "bdr": bdp, "bvec": tables["bvecs"][p],
        })

    res = run_bass_kernel_spmd(nc, in_maps, core_ids=list(range(NCORES)))
    outp = np.empty((N_NODES, N_LABELS), np.float32)
    for p in range(NCORES):
        nodes = np.arange(p * NPC, (p + 1) * NPC)
        outp[nodes] = res.results[p]["out"][pos_all[nodes] % NPAD]
    return outp
